# revision 15
# baseline (speedup 1.0000x reference)
"""Original baseline CQAttention kernel (142µs) kept for A/B device checks."""

from contextlib import ExitStack

import numpy as np

import concourse.bacc as bacc
import concourse.bass as bass
import concourse.mybir as mybir
import concourse.tile as tile
from concourse.bass_utils import run_bass_kernel_spmd
from concourse.masks import make_identity

B, LC, LQ, D = 64, 1024, 128, 256
N_CORES = 8
BL = B // N_CORES
NT = LC // 128
KD = D // 128
F32 = mybir.dt.float32
F32R = mybir.dt.float32r

_CACHE: dict = {}


def _build_bass() -> bass.Bass:
    nc = bacc.Bacc("TRN2")
    C_h = nc.dram_tensor("C", [BL, LC, D], F32, kind="ExternalInput")
    Q_h = nc.dram_tensor("Q", [BL, LQ, D], F32, kind="ExternalInput")
    qm_h = nc.dram_tensor("qmask", [BL, LQ], F32, kind="ExternalInput")
    w_h = nc.dram_tensor("w", [3 * D], F32, kind="ExternalInput")
    out_h = nc.dram_tensor("out", [BL, LC, 2 * D], F32, kind="ExternalOutput")

    with tile.TileContext(nc) as tc, ExitStack() as ctx:
        singles = ctx.enter_context(tc.tile_pool(name="singles", bufs=1))
        c_pool = ctx.enter_context(tc.tile_pool(name="c", bufs=8))
        ct_pool = ctx.enter_context(tc.tile_pool(name="ct", bufs=2))
        e_pool = ctx.enter_context(tc.tile_pool(name="e", bufs=3))
        o_pool = ctx.enter_context(tc.tile_pool(name="o", bufs=3))
        q_pool = ctx.enter_context(tc.tile_pool(name="q", bufs=3))
        tmp_pool = ctx.enter_context(tc.tile_pool(name="tmp", bufs=3))
        small_pool = ctx.enter_context(tc.tile_pool(name="small", bufs=6))
        ctp_pool = ctx.enter_context(tc.tile_pool(name="ctp", bufs=2, space="PSUM"))
        s_pool = ctx.enter_context(tc.tile_pool(name="s", bufs=2, space="PSUM"))
        u_pool = ctx.enter_context(tc.tile_pool(name="u", bufs=4, space="PSUM"))

        pend_load = {}

        def emit_load(b):
            ct = c_pool.tile([128, NT, D], F32R, tag="c", name=f"c{b}")
            nc.sync.dma_start(
                out=ct,
                in_=C_h[b].rearrange("(p t) d -> p t d", t=NT).bitcast(F32R),
            )
            pend_load[b] = ct

        # SP-ring order: all Q-side bytes first (Q-prep compute can start
        # ~11us in), then all 8 C loads stream back-to-back. With bufs=8
        # no load ever waits on a c-pool slot, so the load stream never
        # couples to compute.
        q_all = singles.tile([128, BL, D], F32R)
        nc.sync.dma_start(
            out=q_all,
            in_=bass.AP(
                tensor=Q_h, offset=0, ap=[[D, 128], [LQ * D, BL], [1, D]]
            ).bitcast(F32R),
        )
        qm_all = singles.tile([128, BL], F32)
        nc.sync.dma_start(
            out=qm_all, in_=bass.AP(tensor=qm_h, offset=0, ap=[[1, 128], [LQ, BL]])
        )
        w2rep = singles.tile([128, D], F32)
        nc.sync.dma_start(
            out=w2rep, in_=bass.AP(tensor=w_h, offset=D, ap=[[0, 128], [1, D]])
        )
        w3T = singles.tile([128, KD], F32)
        nc.sync.dma_start(
            out=w3T, in_=bass.AP(tensor=w_h, offset=2 * D, ap=[[1, 128], [128, KD]])
        )
        for _b in range(BL):
            emit_load(_b)

        # f32r identity: PE transposes with f32r operands run at 1.5
        # cycles/row instead of f32's 2.0, and the pass-through is exact.
        # (memset can't target f32r, so build in f32 and round-copy once.)
        ident = singles.tile([128, 128], F32)
        make_identity(nc, ident)
        identr = singles.tile([128, 128], F32R)
        nc.vector.tensor_copy(out=identr, in_=ident)
        ones_col = singles.tile([128, 2], F32)
        nc.vector.memset(ones_col, 1.0)

        q_tiles, q_rnds, qw3Ts, biases = [], [], [], []
        for b in range(BL):
            q_tile = q_all[:, b]
            q_tiles.append(q_tile)
            qm_col = qm_all[:, b : b + 1]
            q_rnd = singles.tile([128, D + 2], F32R, name=f"q_rnd{b}")
            nc.gpsimd.tensor_copy(out=q_rnd[:, :D], in_=q_tile)
            nc.gpsimd.tensor_copy(out=q_rnd[:, D : D + 2], in_=ones_col)
            q_rnds.append(q_rnd)

            qw2 = tmp_pool.tile([128, D], F32, name="qw2")
            nc.vector.tensor_mul(qw2, q_tile.bitcast(F32), w2rep)
            q2 = small_pool.tile([128, 1], F32, name="q2")
            nc.vector.reduce_sum(q2, qw2, axis=mybir.AxisListType.X)
            bias_t = singles.tile([128, 1], F32, name=f"bias{b}")
            nc.vector.tensor_scalar(
                out=bias_t,
                in0=qm_col,
                scalar1=-10000.0,
                scalar2=q2,
                op0=mybir.AluOpType.mult,
                op1=mybir.AluOpType.add,
            )
            biases.append(bias_t)

            qw3T = singles.tile([128, KD, 128], F32R, name=f"qw3T{b}")
            qtp = ctp_pool.tile([128, 256], F32, tag="ctp", name="qtp")
            for k in range(KD):
                nc.tensor.transpose(
                    qtp[:, 128 * k : 128 * (k + 1)].bitcast(F32R),
                    q_tile[:, 128 * k : 128 * (k + 1)],
                    identr,
                )
            for k in range(KD):
                nc.vector.tensor_scalar_mul(
                    out=qw3T[:, k],
                    in0=qtp[:, 128 * k : 128 * (k + 1)],
                    scalar1=w3T[:, k : k + 1],
                )
            qw3Ts.append(qw3T)

        def stage_a(b, c_tile):
            qw3T, bias_t = qw3Ts[b], biases[b]

            ct_tile = ct_pool.tile([128, KD, LC], F32R)
            for t0 in range(0, NT, 2):
                ctp = ctp_pool.tile([128, 512], F32, tag="ctp")
                for dt_ in range(2):
                    for k in range(KD):
                        nc.tensor.transpose(
                            ctp[
                                :, 256 * k + 128 * dt_ : 256 * k + 128 * (dt_ + 1)
                            ].bitcast(F32R),
                            c_tile[:, t0 + dt_, 128 * k : 128 * (k + 1)],
                            identr,
                        )
                src = ctp.rearrange("p (k j) -> p k j", k=KD)
                dst = ct_tile[:, :, 128 * t0 : 128 * (t0 + 2)]
                nc.vector.tensor_copy(out=dst, in_=src)

            s_ps = [
                s_pool.tile([128, 512], F32, tag="s", name=f"s_ps{n}")
                for n in range(2)
            ]
            for k in range(KD):
                for n in range(2):
                    nc.tensor.matmul(
                        s_ps[n],
                        qw3T[:, k],
                        ct_tile[:, k, 512 * n : 512 * (n + 1)],
                        start=(k == 0),
                        stop=(k == KD - 1),
                    )

            e_tile = e_pool.tile([128, LC], F32R)
            for n in range(2):
                nc.scalar.activation(
                    out=e_tile[:, 512 * n : 512 * (n + 1)],
                    in_=s_ps[n],
                    func=mybir.ActivationFunctionType.Exp,
                    bias=bias_t,
                    scale=1.0,
                )
            return c_tile, e_tile

        def stage_b(b, c_tile, e_tile):
            q_rnd = q_rnds[b]
            o_tile = o_pool.tile([128, NT, 2 * D], F32)
            for t in range(NT):
                u_ps = u_pool.tile([128, D + 2], F32, tag="u")
                nc.tensor.matmul(
                    u_ps,
                    e_tile[:, 128 * t : 128 * (t + 1)],
                    q_rnd,
                    start=True,
                    stop=True,
                )
                r_t = small_pool.tile([128, 1], F32)
                nc.vector.reciprocal(out=r_t, in_=u_ps[:, D : D + 1])
                nc.scalar.mul(out=o_tile[:, t, :D], in_=u_ps[:, :D], mul=r_t)
                ca_engine = nc.vector if t % 4 == 1 else nc.gpsimd
                ca_engine.tensor_mul(
                    o_tile[:, t, D:], o_tile[:, t, :D], c_tile[:, t, :].bitcast(F32)
                )

            nc.scalar.dma_start(
                out=out_h[b].rearrange("(p t) f -> p t f", t=NT), in_=o_tile
            )

        pending = {}
        for b in range(BL):
            pending[b] = stage_a(b, pend_load.pop(b))
            if b >= 1:
                stage_b(b - 1, *pending.pop(b - 1))
        stage_b(BL - 1, *pending.pop(BL - 1))
    nc.compile()
    return nc


def _get_bass() -> bass.Bass:
    if "nc" not in _CACHE:
        _CACHE["nc"] = _build_bass()
    return _CACHE["nc"]


def _run(C, Q, qmask, w, trace=False, **spmd_kwargs):
    nc = _get_bass()
    C = np.ascontiguousarray(C, dtype=np.float32)
    Q = np.ascontiguousarray(Q, dtype=np.float32)
    qmask = np.ascontiguousarray(qmask, dtype=np.float32)
    w = np.ascontiguousarray(w, dtype=np.float32)
    in_maps = [
        {
            "C": C[c * BL : (c + 1) * BL],
            "Q": Q[c * BL : (c + 1) * BL],
            "qmask": qmask[c * BL : (c + 1) * BL],
            "w": w,
        }
        for c in range(N_CORES)
    ]
    res = run_bass_kernel_spmd(
        nc, in_maps, list(range(N_CORES)), trace=trace, **spmd_kwargs
    )
    out = np.concatenate([res.results[c]["out"] for c in range(N_CORES)], axis=0)
    return out, res


def kernel(C, Q, cmask, qmask, w):
    out, _ = _run(C, Q, qmask, w, trace=False)
    return out


# revision 16
# speedup vs baseline: 1.0865x; 1.0865x over previous
"""Original baseline CQAttention kernel (142µs) kept for A/B device checks."""

from contextlib import ExitStack

import numpy as np

import concourse.bacc as bacc
import concourse.bass as bass
import concourse.mybir as mybir
import concourse.tile as tile
from concourse.bass_utils import run_bass_kernel_spmd
from concourse.masks import make_identity

B, LC, LQ, D = 64, 1024, 128, 256
N_CORES = 8
BL = B // N_CORES
NT = LC // 128
KD = D // 128
F32 = mybir.dt.float32
F32R = mybir.dt.float32r

_CACHE: dict = {}


def _build_bass() -> bass.Bass:
    nc = bacc.Bacc("TRN2")
    C_h = nc.dram_tensor("C", [BL, LC, D], F32, kind="ExternalInput")
    Q_h = nc.dram_tensor("Q", [BL, LQ, D], F32, kind="ExternalInput")
    qm_h = nc.dram_tensor("qmask", [BL, LQ], F32, kind="ExternalInput")
    w_h = nc.dram_tensor("w", [3 * D], F32, kind="ExternalInput")
    out_h = nc.dram_tensor("out", [BL, LC, 2 * D], F32, kind="ExternalOutput")

    with tile.TileContext(nc) as tc, ExitStack() as ctx:
        singles = ctx.enter_context(tc.tile_pool(name="singles", bufs=1))
        c_pool = ctx.enter_context(tc.tile_pool(name="c", bufs=8))
        ct_pool = ctx.enter_context(tc.tile_pool(name="ct", bufs=2))
        e_pool = ctx.enter_context(tc.tile_pool(name="e", bufs=3))
        o_pool = ctx.enter_context(tc.tile_pool(name="o", bufs=3))
        q_pool = ctx.enter_context(tc.tile_pool(name="q", bufs=3))
        tmp_pool = ctx.enter_context(tc.tile_pool(name="tmp", bufs=3))
        small_pool = ctx.enter_context(tc.tile_pool(name="small", bufs=6))
        ctp_pool = ctx.enter_context(tc.tile_pool(name="ctp", bufs=2, space="PSUM"))
        s_pool = ctx.enter_context(tc.tile_pool(name="s", bufs=2, space="PSUM"))
        u_pool = ctx.enter_context(tc.tile_pool(name="u", bufs=4, space="PSUM"))

        pend_load = {}

        def emit_load(b):
            ct = c_pool.tile([128, NT, D], F32R, tag="c", name=f"c{b}")
            nc.sync.dma_start(
                out=ct,
                in_=C_h[b].rearrange("(p t) d -> p t d", t=NT).bitcast(F32R),
            )
            pend_load[b] = ct

        # SP-ring order: all Q-side bytes first (Q-prep compute can start
        # ~11us in), then all 8 C loads stream back-to-back. With bufs=8
        # no load ever waits on a c-pool slot, so the load stream never
        # couples to compute.
        # q_rnd_all[j, b, :] = [Q[b, j, :], 1, 1] in fp32r, loaded straight
        # from DRAM (fp32r tag is bit-identical; rounding happens in the PE).
        q_rnd_all = singles.tile([128, BL, D + 2], F32R)
        nc.sync.dma_start(
            out=q_rnd_all[:, :, :D],
            in_=bass.AP(
                tensor=Q_h, offset=0, ap=[[D, 128], [LQ * D, BL], [1, D]]
            ).bitcast(F32R),
        )
        qm_all = singles.tile([128, BL], F32)
        nc.sync.dma_start(
            out=qm_all, in_=bass.AP(tensor=qm_h, offset=0, ap=[[1, 128], [LQ, BL]])
        )
        w2rep = singles.tile([128, D], F32)
        nc.sync.dma_start(
            out=w2rep, in_=bass.AP(tensor=w_h, offset=D, ap=[[0, 128], [1, D]])
        )
        w3T = singles.tile([128, KD], F32)
        nc.sync.dma_start(
            out=w3T, in_=bass.AP(tensor=w_h, offset=2 * D, ap=[[1, 128], [128, KD]])
        )
        for _b in range(BL):
            emit_load(_b)

        # f32r identity: PE transposes with f32r operands run at 1.5
        # cycles/row instead of f32's 2.0, and the pass-through is exact.
        # (memset can't target f32r, so build in f32 and round-copy once.)
        ident = singles.tile([128, 128], F32)
        make_identity(nc, ident)
        identr = singles.tile([128, 128], F32R)
        nc.vector.tensor_copy(out=identr, in_=ident)
        ones_col = singles.tile([128, 2], F32)
        nc.vector.memset(ones_col, 1.0)
        # Warm the ACT engine's Exp table during the load phase instead of on
        # the first real EXP.
        warm = small_pool.tile([128, 2], F32, name="warm")
        nc.scalar.activation(out=warm, in_=ones_col, func=mybir.ActivationFunctionType.Exp)

        # ================= per-batch Q-side prep (emitted just-in-time) ====
        q_rnds, qw3Ts, biases = [], [], []
        for b in range(BL):
            q_rnds.append(q_rnd_all[:, b])
            qw3Ts.append(singles.tile([128, KD, 128], F32R, name=f"qw3T{b}"))
            biases.append(singles.tile([128, 1], F32, name=f"bias{b}"))

        def emit_qprep(b):
            q_tile = q_rnd_all[:, b, :D]
            # ones columns for the U' denominator matmul
            nc.vector.tensor_copy(out=q_rnd_all[:, b, D : D + 2], in_=ones_col)

            # bias = Q@w2 - 1e4*qmask, per partition j
            qw2 = tmp_pool.tile([128, D], F32, name="qw2")
            nc.vector.tensor_mul(qw2, q_tile.bitcast(F32), w2rep)
            q2 = small_pool.tile([128, 1], F32, name="q2")
            nc.vector.reduce_sum(q2, qw2, axis=mybir.AxisListType.X)
            nc.vector.tensor_scalar(
                out=biases[b],
                in0=qm_all[:, b : b + 1],
                scalar1=-10000.0,
                scalar2=q2,
                op0=mybir.AluOpType.mult,
                op1=mybir.AluOpType.add,
            )

            # qw3T[k] = (Q^T chunk k) * w3[k] (lhsT of the S matmul)
            qtp = ctp_pool.tile([128, 256], F32, tag="ctp", name="qtp")
            for k in range(KD):
                nc.tensor.transpose(
                    qtp[:, 128 * k : 128 * (k + 1)].bitcast(F32R),
                    q_tile[:, 128 * k : 128 * (k + 1)],
                    identr,
                )
            for k in range(KD):
                nc.vector.tensor_scalar_mul(
                    out=qw3Ts[b][:, k],
                    in0=qtp[:, 128 * k : 128 * (k + 1)],
                    scalar1=w3T[:, k : k + 1],
                )

        emit_qprep(0)
        emit_qprep(1)

        def stage_a(b, c_tile):
            qw3T, bias_t = qw3Ts[b], biases[b]

            ct_tile = ct_pool.tile([128, KD, LC], F32R)
            for t0 in range(0, NT, 2):
                ctp = ctp_pool.tile([128, 512], F32, tag="ctp")
                for dt_ in range(2):
                    for k in range(KD):
                        nc.tensor.transpose(
                            ctp[
                                :, 256 * k + 128 * dt_ : 256 * k + 128 * (dt_ + 1)
                            ].bitcast(F32R),
                            c_tile[:, t0 + dt_, 128 * k : 128 * (k + 1)],
                            identr,
                        )
                src = ctp.rearrange("p (k j) -> p k j", k=KD)
                dst = ct_tile[:, :, 128 * t0 : 128 * (t0 + 2)]
                nc.vector.tensor_copy(out=dst, in_=src)

            s_ps = [
                s_pool.tile([128, 512], F32, tag="s", name=f"s_ps{n}")
                for n in range(2)
            ]
            for k in range(KD):
                for n in range(2):
                    nc.tensor.matmul(
                        s_ps[n],
                        qw3T[:, k],
                        ct_tile[:, k, 512 * n : 512 * (n + 1)],
                        start=(k == 0),
                        stop=(k == KD - 1),
                    )

            e_tile = e_pool.tile([128, LC], F32R)
            for n in range(2):
                nc.scalar.activation(
                    out=e_tile[:, 512 * n : 512 * (n + 1)],
                    in_=s_ps[n],
                    func=mybir.ActivationFunctionType.Exp,
                    bias=bias_t,
                    scale=1.0,
                )
            return c_tile, e_tile

        def stage_b(b, c_tile, e_tile):
            q_rnd = q_rnds[b]
            o_tile = o_pool.tile([128, NT, 2 * D], F32)
            for t in range(NT):
                u_ps = u_pool.tile([128, D + 2], F32, tag="u")
                nc.tensor.matmul(
                    u_ps,
                    e_tile[:, 128 * t : 128 * (t + 1)],
                    q_rnd,
                    start=True,
                    stop=True,
                )
                r_t = small_pool.tile([128, 1], F32)
                nc.vector.reciprocal(out=r_t, in_=u_ps[:, D : D + 1])
                nc.scalar.mul(out=o_tile[:, t, :D], in_=u_ps[:, :D], mul=r_t)
                ca_engine = nc.vector if t % 2 == 1 else nc.gpsimd
                ca_engine.tensor_mul(
                    o_tile[:, t, D:], o_tile[:, t, :D], c_tile[:, t, :].bitcast(F32)
                )

            nc.scalar.dma_start(
                out=out_h[b].rearrange("(p t) f -> p t f", t=NT), in_=o_tile
            )

        pending = {}
        for b in range(BL):
            pending[b] = stage_a(b, pend_load.pop(b))
            if b + 2 < BL:
                emit_qprep(b + 2)
            if b >= 1:
                stage_b(b - 1, *pending.pop(b - 1))
        stage_b(BL - 1, *pending.pop(BL - 1))
    nc.compile()
    return nc


def _get_bass() -> bass.Bass:
    if "nc" not in _CACHE:
        _CACHE["nc"] = _build_bass()
    return _CACHE["nc"]


def _run(C, Q, qmask, w, trace=False, **spmd_kwargs):
    nc = _get_bass()
    C = np.ascontiguousarray(C, dtype=np.float32)
    Q = np.ascontiguousarray(Q, dtype=np.float32)
    qmask = np.ascontiguousarray(qmask, dtype=np.float32)
    w = np.ascontiguousarray(w, dtype=np.float32)
    in_maps = [
        {
            "C": C[c * BL : (c + 1) * BL],
            "Q": Q[c * BL : (c + 1) * BL],
            "qmask": qmask[c * BL : (c + 1) * BL],
            "w": w,
        }
        for c in range(N_CORES)
    ]
    res = run_bass_kernel_spmd(
        nc, in_maps, list(range(N_CORES)), trace=trace, **spmd_kwargs
    )
    out = np.concatenate([res.results[c]["out"] for c in range(N_CORES)], axis=0)
    return out, res


def kernel(C, Q, cmask, qmask, w):
    out, _ = _run(C, Q, qmask, w, trace=False)
    return out
